# revision 3
# baseline (speedup 1.0000x reference)
"""Per-core causal multi-head attention Bass/Tile program builder.

One core handles: batch b, one head-group (DH of the model's head dims).
Computation (all on-chip after the initial loads, fp32r matmuls):
  qT = wqT.T @ xT          [DH, S]   (head dims on partitions)
  kT = wkT.T @ xT          [DH, S]
  v  = xT.T @ wvT          [S, DH]   (+ a ones column per head for softmax denom)
  per head pair p (2 heads stacked on 128 partitions):
    scoresT[sk, q] = kT.T @ qT    (K=64 contraction per head, heads packed in
                                   row strips 0-63 / 64-127 of the PE array)
    expT = exp(0.125 * scoresT)   (ACT, one [128,1024] op for the pair)
    expT *= causal mask           (diagonal tiles only)
    avT[65, q] += [v|1].T @ expT  (row 64 accumulates the softmax denominator)
    avT[0:64] *= 1/denom          (K=1 ones-matmul broadcasts the reciprocal row)
  out[s, :] = avT.T @ woT       (accumulated over head pairs, written to HBM)
"""

from contextlib import ExitStack

import numpy as np

import concourse.bass as bass
import concourse.bacc as bacc
import concourse.mybir as mybir
import concourse.tile as tile

F32 = mybir.dt.float32
F32R = mybir.dt.float32r


def r(ap):
    """View an fp32 AP as float32r for full-rate PE matmuls."""
    return ap.bitcast(F32R)


def make_masks(n_j=4, qb=512, extra_ones=64):
    """[128, n_j*qb + extra_ones] fp32: causal 0/1 masks for the n_j diagonal
    block offsets, plus a strip of ones (bcast-matmul lhsT / v ones source)."""
    p = np.arange(128)[:, None]
    f = np.arange(qb)[None, :]
    cols = [((p + 128 * j) <= f).astype(np.float32) for j in range(n_j)]
    cols.append(np.ones((128, extra_ones), np.float32))
    return np.concatenate(cols, axis=1)


def build_core_program(S=2048, D=1024, DH=512, DOUT=1024, QB=512, debug=False):
    """Build the per-core Bass program. Returns nc."""
    P = 128
    HP = DH // P            # head pairs
    H = DH // 64            # heads on this core
    ND = D // P             # d tiles
    NS = S // P             # s tiles of 128
    NQB = S // QB           # q blocks
    NSB = S // 512          # s blocks of 512 (projection free blocks)
    NJ = QB // P            # diagonal offsets per q block
    MCOLS = NJ * QB + 64    # masks input width

    nc = bacc.Bacc()

    xT = nc.dram_tensor("xT", [D, S], F32R, kind="ExternalInput")
    wqT = nc.dram_tensor("wqT", [D, DH], F32R, kind="ExternalInput")
    wkT = nc.dram_tensor("wkT", [D, DH], F32R, kind="ExternalInput")
    wvT = nc.dram_tensor("wvT", [D, DH], F32R, kind="ExternalInput")
    woT = nc.dram_tensor("woT", [DH, DOUT], F32R, kind="ExternalInput")
    masks = nc.dram_tensor("masks", [P, MCOLS], F32R, kind="ExternalInput")
    out = nc.dram_tensor("out", [S, DOUT], F32, kind="ExternalOutput")
    if debug:
        dbg_qT = nc.dram_tensor("dbg_qT", [P, S], F32, kind="ExternalOutput")
        dbg_kT = nc.dram_tensor("dbg_kT", [P, S], F32, kind="ExternalOutput")
        dbg_v = nc.dram_tensor("dbg_v", [P, H * 65], F32, kind="ExternalOutput")
        dbg_avT = nc.dram_tensor("dbg_avT", [P, S], F32, kind="ExternalOutput")
        dbg_ex = nc.dram_tensor("dbg_ex", [P, 1024], F32, kind="ExternalOutput")

    lp = nc.allow_low_precision(reason="float32r is bitwise float32 on every non-PE datapath")
    with lp, tile.TileContext(nc) as tc, ExitStack() as ctx:
        const_pool = ctx.enter_context(tc.tile_pool(name="const", bufs=1))
        # x half-tiles and avT share slots (x is dead before avT allocates)
        big_pool = ctx.enter_context(tc.tile_pool(name="big", bufs=ND // 2 + 1))
        qk_pool = ctx.enter_context(tc.tile_pool(name="qk", bufs=2 * HP))
        v_pool = ctx.enter_context(tc.tile_pool(name="v", bufs=NS))
        w_pool = ctx.enter_context(tc.tile_pool(name="w", bufs=ND + 2))
        e_pool = ctx.enter_context(tc.tile_pool(name="e1024", bufs=2 + HP))
        o_pool = ctx.enter_context(tc.tile_pool(name="outp", bufs=3))
        # one PSUM pool, 8 banks total: ps 2 + sc 2x2 + av 2 = 8 (bc shares "ps")
        psum_pool = ctx.enter_context(tc.tile_pool(name="psum", bufs=1, space="PSUM"))

        mask_t = const_pool.tile([P, MCOLS], F32R, tag="masks")
        nc.sync.dma_start(mask_t[:], masks[:, :])
        ones64 = mask_t[:, NJ * QB : NJ * QB + 64]  # all-ones [128, 64]

        # persistent activations
        qT = [qk_pool.tile([P, S], F32R, tag="qk", name="qT") for _ in range(HP)]
        kT = [qk_pool.tile([P, S], F32R, tag="qk", name="kT") for _ in range(HP)]
        v_t = [v_pool.tile([P, H * 65], F32R, tag="v", name="v_t") for _ in range(NS)]
        woT_t = [e_pool.tile([P, DOUT], F32R, tag="e1024", name="woT_t") for _ in range(HP)]
        for p in range(HP):
            nc.sync.dma_start(woT_t[p][:], woT[p * P : (p + 1) * P, :])

        # ---- projections, d contracted in two halves to bound SBUF ----
        HALF = ND // 2
        for half in range(2):
            x_t = [big_pool.tile([P, S], F32R, tag="big", name="x_t") for _ in range(HALF)]
            for i in range(HALF):
                dt = half * HALF + i
                nc.sync.dma_start(x_t[i][:], xT[dt * P : (dt + 1) * P, :])

            wq_t = [w_pool.tile([P, DH], F32R, tag="w", name="wq_t") for _ in range(HALF)]
            wk_t = [w_pool.tile([P, DH], F32R, tag="w", name="wk_t") for _ in range(HALF)]
            wv_t = [w_pool.tile([P, DH], F32R, tag="w", name="wv_t") for _ in range(HALF)]
            for i in range(HALF):
                dt = half * HALF + i
                nc.sync.dma_start(wq_t[i][:], wqT[dt * P : (dt + 1) * P, :])
                nc.sync.dma_start(wk_t[i][:], wkT[dt * P : (dt + 1) * P, :])
                nc.sync.dma_start(wv_t[i][:], wvT[dt * P : (dt + 1) * P, :])

            # q/k: out [dh-pair 128, s-block 512]
            for w_t, dst in ((wq_t, qT), (wk_t, kT)):
                for p in range(HP):
                    for sb in range(NSB):
                        ps = psum_pool.tile([P, 512], F32, tag="ps", bufs=2)
                        for i in range(HALF):
                            nc.tensor.matmul(
                                ps[:],
                                r(w_t[i][:, p * P : (p + 1) * P]),
                                r(x_t[i][:, sb * 512 : (sb + 1) * 512]),
                                start=(i == 0),
                                stop=(i == HALF - 1),
                            )
                        sl = dst[p][:, sb * 512 : (sb + 1) * 512]
                        if half == 0:
                            nc.vector.tensor_copy(sl, ps[:])
                        else:
                            nc.vector.tensor_add(sl, sl, ps[:])

            # v: out [s-tile 128, DH] -> strided per-head (64 cols + ones col)
            for st in range(NS):
                ps = psum_pool.tile([P, 512], F32, tag="ps", name="ps_v", bufs=2)[:, :DH]
                for i in range(HALF):
                    nc.tensor.matmul(
                        ps[:],
                        r(x_t[i][:, st * P : (st + 1) * P]),
                        r(wv_t[i][:]),
                        start=(i == 0),
                        stop=(i == HALF - 1),
                    )
                dst = v_t[st][:].rearrange("p (h c) -> p h c", c=65)[:, :, 0:64]
                src = ps[:].rearrange("p (h c) -> p h c", c=64)
                if half == 0:
                    nc.vector.tensor_copy(dst, src)
                else:
                    nc.vector.tensor_add(dst, dst, src)
            if half == 1:
                for st in range(NS):
                    onescol = v_t[st][:].rearrange("p (h c) -> p h c", c=65)[:, :, 64:65]
                    nc.vector.tensor_copy(
                        onescol, ones64[:, 0:H].rearrange("p (h c) -> p h c", c=1)
                    )

        # ---- attention ----
        avT = [big_pool.tile([P, S], F32R, tag="big", name="avT") for _ in range(HP)]
        for p in range(HP):
            hA, hB = 2 * p, 2 * p + 1
            for qb in range(NQB):
                Q0 = qb * QB
                av_ps = [psum_pool.tile([65, 512], F32, tag="av", name="av_ps", bufs=2) for _ in range(2)]
                nsk = (Q0 + QB) // P
                for sk in range(nsk):
                    K0 = sk * P
                    sc = psum_pool.tile([P, 1024], F32, tag="sc", bufs=2)
                    nc.tensor.matmul(
                        sc[:, 0:512],
                        r(kT[p][0:64, K0 : K0 + P]),
                        r(qT[p][0:64, Q0 : Q0 + QB]),
                    )
                    nc.tensor.matmul(
                        sc[:, 512:1024],
                        r(kT[p][64:128, K0 : K0 + P]),
                        r(qT[p][64:128, Q0 : Q0 + QB]),
                    )
                    ex = e_pool.tile([P, 1024], F32R, tag="e1024")
                    nc.scalar.activation(
                        ex[:], sc[:], mybir.ActivationFunctionType.Exp, scale=0.125
                    )
                    if debug and p == 0 and qb == 0 and sk == 0:
                        nc.sync.dma_start(dbg_ex[:, :], ex[:])
                    j = sk - NJ * qb
                    if j >= 0:  # diagonal block: apply causal mask
                        m = mask_t[:, j * QB : j * QB + QB]
                        nc.vector.tensor_mul(ex[:, 0:512], ex[:, 0:512], m)
                        nc.vector.tensor_mul(ex[:, 512:1024], ex[:, 512:1024], m)
                    for hi, h in enumerate((hA, hB)):
                        nc.tensor.matmul(
                            av_ps[hi][:],
                            r(v_t[sk][:, h * 65 : h * 65 + 65]),
                            r(ex[:, hi * 512 : hi * 512 + 512]),
                            start=(sk == 0),
                            stop=(sk == nsk - 1),
                        )
                # softmax normalization: row 64 of av_ps holds the denominator
                for hi in range(2):
                    rc = o_pool.tile([P, 512], F32R, tag="outp")
                    nc.vector.reciprocal(rc[64:65, :], av_ps[hi][64:65, :])
                    bc = psum_pool.tile([64, 512], F32, tag="ps", bufs=2)
                    nc.tensor.matmul(bc[:], r(ones64[64:65, :]), r(rc[64:65, :]))
                    bcs = o_pool.tile([P, 512], F32, tag="outp")
                    nc.scalar.copy(bcs[0:64, :], bc[:])
                    if hi == 0:
                        nc.vector.tensor_mul(
                            avT[p][0:64, Q0 : Q0 + QB], av_ps[hi][0:64, :], bcs[0:64, :]
                        )
                    else:
                        tmp = o_pool.tile([P, 512], F32R, tag="outp")
                        nc.vector.tensor_mul(tmp[0:64, :], av_ps[hi][0:64, :], bcs[0:64, :])
                        # partition shift 0:64 -> 64:128 (engines can't cross lanes)
                        nc.sync.dma_start(avT[p][64:128, Q0 : Q0 + QB], tmp[0:64, :])

        if debug:
            nc.sync.dma_start(dbg_qT[:, :], qT[0][:])
            nc.sync.dma_start(dbg_kT[:, :], kT[0][:])
            nc.sync.dma_start(dbg_v[:, :], v_t[0][:])
            nc.sync.dma_start(dbg_avT[:, :], avT[0][:])

        # ---- output projection: out[s, n] = sum_p avT[p].T @ woT[p] ----
        NW = min(512, DOUT)
        for st in range(NS):
            for nb in range(DOUT // NW):
                ps = psum_pool.tile([P, 512], F32, tag="ps", name="ps_o", bufs=2)
                for p in range(HP):
                    nc.tensor.matmul(
                        ps[:, :NW],
                        r(avT[p][:, st * P : (st + 1) * P]),
                        r(woT_t[p][:, nb * NW : (nb + 1) * NW]),
                        start=(p == 0),
                        stop=(p == HP - 1),
                    )
                ot = o_pool.tile([P, 512], F32, tag="outp", name="ot")
                nc.vector.tensor_copy(ot[:, :NW], ps[:, :NW])
                nc.sync.dma_start(
                    out[st * P : (st + 1) * P, nb * NW : (nb + 1) * NW], ot[:, :NW]
                )

    nc.compile()
    return nc


def shard_inputs(x, wq, wk, wv, wo, n_cores=8):
    """Full inputs -> per-core in_maps. Core c: batch c//2, head-group c%2."""
    B = x.shape[0]
    D = wq.shape[1]
    hg_w = wq.shape[0] // (n_cores // B)
    masks = make_masks()
    in_maps = []
    for c in range(n_cores):
        b, hg = c // (n_cores // B), c % (n_cores // B)
        sl = slice(hg * hg_w, (hg + 1) * hg_w)
        in_maps.append(
            {
                "xT": np.ascontiguousarray(x[b].T),
                "wqT": np.ascontiguousarray(wq[sl, :].T),
                "wkT": np.ascontiguousarray(wk[sl, :].T),
                "wvT": np.ascontiguousarray(wv[sl, :].T),
                "woT": np.ascontiguousarray(wo[:, sl].T),
                "masks": masks,
            }
        )
    return in_maps


def unshard_outputs(results, B=4):
    """Per-core 'out' partials -> full [B, S, D] output (sum head-group pairs)."""
    per_b = len(results) // B
    outs = []
    for b in range(B):
        acc = results[b * per_b]["out"].astype(np.float32)
        for i in range(1, per_b):
            acc = acc + results[b * per_b + i]["out"]
        outs.append(acc)
    return np.stack(outs, axis=0)


# ---------------------------------------------------------------------------
# Full-kernel entry point: FULL inputs -> FULL output, 8 NeuronCores.
# Sharding: core c -> (batch c//2, head-group c%2). Each core computes its
# batch's attention for 8 of the 16 heads plus that head-group's slice of the
# output projection; the two partial outputs per batch are summed on the host
# (the unshard step of the tensor-parallel split of wo).
# ---------------------------------------------------------------------------

_NC_CACHE = {}


def _get_program():
    if "nc" not in _NC_CACHE:
        _NC_CACHE["nc"] = build_core_program(S=2048, D=1024, DH=512, DOUT=1024)
    return _NC_CACHE["nc"]


def kernel(x, wq, wk, wv, wo):
    from concourse import bass_utils

    x = np.asarray(x, dtype=np.float32)
    wq = np.asarray(wq, dtype=np.float32)
    wk = np.asarray(wk, dtype=np.float32)
    wv = np.asarray(wv, dtype=np.float32)
    wo = np.asarray(wo, dtype=np.float32)

    nc = _get_program()
    in_maps = shard_inputs(x, wq, wk, wv, wo, n_cores=8)
    res = bass_utils.run_bass_kernel_spmd(nc, in_maps, core_ids=list(range(8)))
    return unshard_outputs(res.results, B=x.shape[0])


# revision 4
# speedup vs baseline: 1.1094x; 1.1094x over previous
"""Per-core causal multi-head attention Bass/Tile program builder.

One core handles: batch b, one head-group (DH of the model's head dims).
Computation (all on-chip after the initial loads, fp32r matmuls):
  qT = wqT.T @ xT          [DH, S]   (head dims on partitions)
  kT = wkT.T @ xT          [DH, S]
  v  = xT.T @ wvT          [S, DH]   (+ a ones column per head for softmax denom)
  per head pair p (2 heads stacked on 128 partitions):
    scoresT[sk, q] = kT.T @ qT    (K=64 contraction per head, heads packed in
                                   row strips 0-63 / 64-127 of the PE array)
    expT = exp(0.125 * scoresT)   (ACT, one [128,1024] op for the pair)
    expT *= causal mask           (diagonal tiles only)
    avT[65, q] += [v|1].T @ expT  (row 64 accumulates the softmax denominator)
    avT[0:64] *= 1/denom          (K=1 ones-matmul broadcasts the reciprocal row)
  out[s, :] = avT.T @ woT       (accumulated over head pairs, written to HBM)
"""

from contextlib import ExitStack

import numpy as np

import concourse.bass as bass
import concourse.bacc as bacc
import concourse.mybir as mybir
import concourse.tile as tile

F32 = mybir.dt.float32
F32R = mybir.dt.float32r


def r(ap):
    """View an fp32 AP as float32r for full-rate PE matmuls."""
    return ap.bitcast(F32R)


def make_masks(n_j=4, qb=512, extra_ones=64):
    """[128, n_j*qb + extra_ones] fp32: causal 0/1 masks for the n_j diagonal
    block offsets, plus a strip of ones (bcast-matmul lhsT / v ones source)."""
    p = np.arange(128)[:, None]
    f = np.arange(qb)[None, :]
    cols = [((p + 128 * j) <= f).astype(np.float32) for j in range(n_j)]
    cols.append(np.ones((128, extra_ones), np.float32))
    return np.concatenate(cols, axis=1)


def build_core_program(S=2048, D=1024, DH=512, DOUT=1024, QB=512, debug=False):
    """Build the per-core Bass program. Returns nc."""
    P = 128
    HP = DH // P            # head pairs
    H = DH // 64            # heads on this core
    ND = D // P             # d tiles
    NS = S // P             # s tiles of 128
    NQB = S // QB           # q blocks
    NSB = S // 512          # s blocks of 512 (projection free blocks)
    NJ = QB // P            # diagonal offsets per q block
    MCOLS = NJ * QB + 64    # masks input width

    nc = bacc.Bacc()

    xT = nc.dram_tensor("xT", [D, S], F32R, kind="ExternalInput")
    wqT = nc.dram_tensor("wqT", [D, DH], F32R, kind="ExternalInput")
    wkT = nc.dram_tensor("wkT", [D, DH], F32R, kind="ExternalInput")
    wvT = nc.dram_tensor("wvT", [D, DH], F32R, kind="ExternalInput")
    woT = nc.dram_tensor("woT", [DH, DOUT], F32R, kind="ExternalInput")
    masks = nc.dram_tensor("masks", [P, MCOLS], F32R, kind="ExternalInput")
    out = nc.dram_tensor("out", [S, DOUT], F32, kind="ExternalOutput")
    if debug:
        dbg_qT = nc.dram_tensor("dbg_qT", [P, S], F32, kind="ExternalOutput")
        dbg_kT = nc.dram_tensor("dbg_kT", [P, S], F32, kind="ExternalOutput")
        dbg_v = nc.dram_tensor("dbg_v", [P, H * 65], F32, kind="ExternalOutput")
        dbg_avT = nc.dram_tensor("dbg_avT", [P, S], F32, kind="ExternalOutput")
        dbg_ex = nc.dram_tensor("dbg_ex", [P, 1024], F32, kind="ExternalOutput")

    lp = nc.allow_low_precision(reason="float32r is bitwise float32 on every non-PE datapath")
    with lp, tile.TileContext(nc) as tc, ExitStack() as ctx:
        const_pool = ctx.enter_context(tc.tile_pool(name="const", bufs=1))
        # x half-tiles and avT share slots (x is dead before avT allocates)
        big_pool = ctx.enter_context(tc.tile_pool(name="big", bufs=ND // 2 + 1))
        qk_pool = ctx.enter_context(tc.tile_pool(name="qk", bufs=2 * HP))
        v_pool = ctx.enter_context(tc.tile_pool(name="v", bufs=NS))
        w_pool = ctx.enter_context(tc.tile_pool(name="w", bufs=ND + 2))
        e_pool = ctx.enter_context(tc.tile_pool(name="e1024", bufs=2 + HP))
        o_pool = ctx.enter_context(tc.tile_pool(name="outp", bufs=3))
        # one PSUM pool, 8 banks total: ps 2 + sc 2x2 + av 2 = 8 (bc shares "ps")
        psum_pool = ctx.enter_context(tc.tile_pool(name="psum", bufs=1, space="PSUM"))

        mask_t = const_pool.tile([P, MCOLS], F32R, tag="masks")
        nc.sync.dma_start(mask_t[:], masks[:, :])
        ones64 = mask_t[:, NJ * QB : NJ * QB + 64]  # all-ones [128, 64]

        # persistent activations
        qT = [qk_pool.tile([P, S], F32R, tag="qk", name="qT") for _ in range(HP)]
        kT = [qk_pool.tile([P, S], F32R, tag="qk", name="kT") for _ in range(HP)]
        v_t = [v_pool.tile([P, H * 65], F32R, tag="v", name="v_t") for _ in range(NS)]
        woT_t = [e_pool.tile([P, DOUT], F32R, tag="e1024", name="woT_t") for _ in range(HP)]
        for p in range(HP):
            nc.sync.dma_start(woT_t[p][:], woT[p * P : (p + 1) * P, :])

        # ---- projections, d contracted in two halves to bound SBUF ----
        HALF = ND // 2
        for half in range(2):
            x_t = [big_pool.tile([P, S], F32R, tag="big", name="x_t") for _ in range(HALF)]
            for i in range(HALF):
                dt = half * HALF + i
                nc.sync.dma_start(x_t[i][:], xT[dt * P : (dt + 1) * P, :])

            wq_t = [w_pool.tile([P, DH], F32R, tag="w", name="wq_t") for _ in range(HALF)]
            wk_t = [w_pool.tile([P, DH], F32R, tag="w", name="wk_t") for _ in range(HALF)]
            wv_t = [w_pool.tile([P, DH], F32R, tag="w", name="wv_t") for _ in range(HALF)]
            for i in range(HALF):
                dt = half * HALF + i
                nc.sync.dma_start(wq_t[i][:], wqT[dt * P : (dt + 1) * P, :])
                nc.sync.dma_start(wk_t[i][:], wkT[dt * P : (dt + 1) * P, :])
                nc.sync.dma_start(wv_t[i][:], wvT[dt * P : (dt + 1) * P, :])

            # q/k: out [dh-pair 128, s-block 512]
            for w_t, dst in ((wq_t, qT), (wk_t, kT)):
                for p in range(HP):
                    for sb in range(NSB):
                        ps = psum_pool.tile([P, 512], F32, tag="ps", bufs=2)
                        for i in range(HALF):
                            nc.tensor.matmul(
                                ps[:],
                                r(w_t[i][:, p * P : (p + 1) * P]),
                                r(x_t[i][:, sb * 512 : (sb + 1) * 512]),
                                start=(i == 0),
                                stop=(i == HALF - 1),
                            )
                        sl = dst[p][:, sb * 512 : (sb + 1) * 512]
                        if half == 0:
                            nc.vector.tensor_copy(sl, ps[:])
                        else:
                            nc.vector.tensor_add(sl, sl, ps[:])

            # v: out [s-tile 128, DH] -> strided per-head (64 cols + ones col)
            for st in range(NS):
                ps = psum_pool.tile([P, 512], F32, tag="ps", name="ps_v", bufs=2)[:, :DH]
                for i in range(HALF):
                    nc.tensor.matmul(
                        ps[:],
                        r(x_t[i][:, st * P : (st + 1) * P]),
                        r(wv_t[i][:]),
                        start=(i == 0),
                        stop=(i == HALF - 1),
                    )
                dst = v_t[st][:].rearrange("p (h c) -> p h c", c=65)[:, :, 0:64]
                src = ps[:].rearrange("p (h c) -> p h c", c=64)
                if half == 0:
                    nc.vector.tensor_copy(dst, src)
                else:
                    nc.vector.tensor_add(dst, dst, src)
            if half == 1:
                for st in range(NS):
                    onescol = v_t[st][:].rearrange("p (h c) -> p h c", c=65)[:, :, 64:65]
                    nc.vector.tensor_copy(
                        onescol, ones64[:, 0:H].rearrange("p (h c) -> p h c", c=1)
                    )

        # ---- attention ----
        avT = [big_pool.tile([P, S], F32R, tag="big", name="avT") for _ in range(HP)]
        for p in range(HP):
            hA, hB = 2 * p, 2 * p + 1
            for qb in range(NQB):
                Q0 = qb * QB
                av_ps = [psum_pool.tile([65, 512], F32, tag="av", name="av_ps", bufs=2) for _ in range(2)]
                nsk = (Q0 + QB) // P
                for sk in range(nsk):
                    K0 = sk * P
                    j = sk - NJ * qb  # >= 0 on diagonal blocks
                    # causal: q columns f < 128j of this block can't attend
                    # to this k tile — skip them in scores/exp/av entirely.
                    c0 = max(0, j) * P
                    sc = psum_pool.tile([P, 1024], F32, tag="sc", bufs=2)
                    for hi in range(2):
                        nc.tensor.matmul(
                            sc[:, hi * 512 + c0 : hi * 512 + 512],
                            r(kT[p][hi * 64 : hi * 64 + 64, K0 : K0 + P]),
                            r(qT[p][hi * 64 : hi * 64 + 64, Q0 + c0 : Q0 + QB]),
                        )
                    ex = e_pool.tile([P, 1024], F32R, tag="e1024")
                    for hi in range(2):
                        nc.scalar.activation(
                            ex[:, hi * 512 + c0 : hi * 512 + 512],
                            sc[:, hi * 512 + c0 : hi * 512 + 512],
                            mybir.ActivationFunctionType.Exp,
                            scale=0.125,
                        )
                    if j >= 0:  # diagonal strip [c0, c0+128): triangular mask
                        m128 = mask_t[:, 0:P]
                        for hi in range(2):
                            nc.vector.tensor_mul(
                                ex[:, hi * 512 + c0 : hi * 512 + c0 + P],
                                ex[:, hi * 512 + c0 : hi * 512 + c0 + P],
                                m128,
                            )
                    for hi, h in enumerate((hA, hB)):
                        nc.tensor.matmul(
                            av_ps[hi][:, c0:512],
                            r(v_t[sk][:, h * 65 : h * 65 + 65]),
                            r(ex[:, hi * 512 + c0 : hi * 512 + 512]),
                            start=(sk == 0),
                            stop=(sk == nsk - 1),
                        )
                # softmax normalization: row 64 of av_ps holds the denominator.
                # broadcast denom across 64 lanes with a K=1 matmul, then one
                # fast-approx reciprocal (5x cheaper than exact) and multiply.
                for hi in range(2):
                    dn = o_pool.tile([P, 512], F32R, tag="outp", name="dn")
                    nc.vector.tensor_copy(dn[64:65, :], av_ps[hi][64:65, :])
                    bc = psum_pool.tile([64, 512], F32, tag="ps", name="bc", bufs=2)
                    nc.tensor.matmul(bc[:], r(ones64[64:65, :]), r(dn[64:65, :]))
                    rcb = o_pool.tile([P, 512], F32, tag="outp", name="rcb")
                    nc.vector.reciprocal_approx_fast(out=rcb[0:64, :], in_=bc[:])
                    if hi == 0:
                        nc.vector.tensor_mul(
                            avT[p][0:64, Q0 : Q0 + QB], av_ps[hi][0:64, :], rcb[0:64, :]
                        )
                    else:
                        tmp = o_pool.tile([P, 512], F32R, tag="outp")
                        nc.vector.tensor_mul(tmp[0:64, :], av_ps[hi][0:64, :], rcb[0:64, :])
                        # partition shift 0:64 -> 64:128 (engines can't cross lanes)
                        nc.sync.dma_start(avT[p][64:128, Q0 : Q0 + QB], tmp[0:64, :])

        if debug:
            nc.sync.dma_start(dbg_qT[:, :], qT[0][:])
            nc.sync.dma_start(dbg_kT[:, :], kT[0][:])
            nc.sync.dma_start(dbg_v[:, :], v_t[0][:])
            nc.sync.dma_start(dbg_avT[:, :], avT[0][:])

        # ---- output projection: out[s, n] = sum_p avT[p].T @ woT[p] ----
        NW = min(512, DOUT)
        for st in range(NS):
            for nb in range(DOUT // NW):
                ps = psum_pool.tile([P, 512], F32, tag="ps", name="ps_o", bufs=2)
                for p in range(HP):
                    nc.tensor.matmul(
                        ps[:, :NW],
                        r(avT[p][:, st * P : (st + 1) * P]),
                        r(woT_t[p][:, nb * NW : (nb + 1) * NW]),
                        start=(p == 0),
                        stop=(p == HP - 1),
                    )
                ot = o_pool.tile([P, 512], F32, tag="outp", name="ot")
                nc.vector.tensor_copy(ot[:, :NW], ps[:, :NW])
                nc.sync.dma_start(
                    out[st * P : (st + 1) * P, nb * NW : (nb + 1) * NW], ot[:, :NW]
                )

    nc.compile()
    return nc


def shard_inputs(x, wq, wk, wv, wo, n_cores=8):
    """Full inputs -> per-core in_maps. Core c: batch c//2, head-group c%2."""
    B = x.shape[0]
    D = wq.shape[1]
    hg_w = wq.shape[0] // (n_cores // B)
    masks = make_masks()
    in_maps = []
    for c in range(n_cores):
        b, hg = c // (n_cores // B), c % (n_cores // B)
        sl = slice(hg * hg_w, (hg + 1) * hg_w)
        in_maps.append(
            {
                "xT": np.ascontiguousarray(x[b].T),
                "wqT": np.ascontiguousarray(wq[sl, :].T),
                "wkT": np.ascontiguousarray(wk[sl, :].T),
                "wvT": np.ascontiguousarray(wv[sl, :].T),
                "woT": np.ascontiguousarray(wo[:, sl].T),
                "masks": masks,
            }
        )
    return in_maps


def unshard_outputs(results, B=4):
    """Per-core 'out' partials -> full [B, S, D] output (sum head-group pairs)."""
    per_b = len(results) // B
    outs = []
    for b in range(B):
        acc = results[b * per_b]["out"].astype(np.float32)
        for i in range(1, per_b):
            acc = acc + results[b * per_b + i]["out"]
        outs.append(acc)
    return np.stack(outs, axis=0)


# ---------------------------------------------------------------------------
# Full-kernel entry point: FULL inputs -> FULL output, 8 NeuronCores.
# Sharding: core c -> (batch c//2, head-group c%2). Each core computes its
# batch's attention for 8 of the 16 heads plus that head-group's slice of the
# output projection; the two partial outputs per batch are summed on the host
# (the unshard step of the tensor-parallel split of wo).
# ---------------------------------------------------------------------------

_NC_CACHE = {}


def _get_program():
    if "nc" not in _NC_CACHE:
        _NC_CACHE["nc"] = build_core_program(S=2048, D=1024, DH=512, DOUT=1024)
    return _NC_CACHE["nc"]


def kernel(x, wq, wk, wv, wo):
    from concourse import bass_utils

    x = np.asarray(x, dtype=np.float32)
    wq = np.asarray(wq, dtype=np.float32)
    wk = np.asarray(wk, dtype=np.float32)
    wv = np.asarray(wv, dtype=np.float32)
    wo = np.asarray(wo, dtype=np.float32)

    nc = _get_program()
    in_maps = shard_inputs(x, wq, wk, wv, wo, n_cores=8)
    res = bass_utils.run_bass_kernel_spmd(nc, in_maps, core_ids=list(range(8)))
    return unshard_outputs(res.results, B=x.shape[0])
